# revision 23
# baseline (speedup 1.0000x reference)
"""Euclidean distance layer (retrieval kNN) on 8 Trainium2 NeuronCores.

out[b, o] = || x[b, :] - weight[:, o] ||_2   for x [2048, 1024], weight [1024, 16384].

Strategy (sharding_hint): shard output columns across the 8 cores (2048 each).
Per core, compute d2 = x2[b] + w2[o] - 2 * (x @ w_shard) and out = sqrt(d2):
  - the big matmul runs in fp8e4 with DoubleRow perf mode (2 MACs/cell/cycle,
    8x the fp32 rate; its rounding is attenuated ~64x in the output because
    |2xw| << d2); each instruction contracts a pair of K=128 tiles
  - every PSUM accumulation group is seeded with -w2/2 broadcast to all
    partitions by a DoubleRow ones-matmul against a [(-w2/2); 0] fp8 row pair,
    so the epilogue needs no elementwise add
  - w2 = colsum(w^2) itself comes from a (-1/2)-constant stationary matmul
    over bf16 squares (reduction + partition-broadcast in one PE op)
  - x2 = rowsum(x^2) is one DVE tensor_tensor_reduce per row tile on an fp16
    copy of x
  - epilogue per [128, 512] tile is a single ACT sqrt(-2*psum + x2_bias)
Host side only transposes/shards/casts inputs and reassembles the output.
"""
import numpy as np

import concourse.bass as bass
import concourse.tile as tile
from concourse import bacc, mybir
from concourse.bass_utils import run_bass_kernel_spmd

f32 = mybir.dt.float32
f32r = mybir.dt.float32r
f16 = mybir.dt.float16
bf16 = mybir.dt.bfloat16
AF = mybir.ActivationFunctionType

B = 2048      # batch rows
I = 1024      # input size (contraction)
O = 16384     # output size (prototype count)
N_CORES = 8
OS = O // N_CORES   # 2048 output columns per core
P = 128       # partitions
NB = 512      # moving free-dim per matmul / psum bank
KT = I // P   # 8 k-tiles
MT = B // P   # 16 m-tiles
NT = OS // NB  # 4 n-blocks

fp8 = mybir.dt.float8e4
MM_DT = fp8           # matmul input dtype: fp8 (DoubleRow), bf16, or f32r
DR = mybir.MatmulPerfMode.DoubleRow if MM_DT is fp8 else None


def _emit_body(nc, tc, x_d, xt_d, w_d, out_d):
    from contextlib import ExitStack
    with ExitStack() as ctx:
        const_p = ctx.enter_context(tc.tile_pool(name="const", bufs=1))
        xt_p = ctx.enter_context(tc.tile_pool(name="xt", bufs=1))
        w_p = ctx.enter_context(tc.tile_pool(name="w", bufs=1))
        xr_p = ctx.enter_context(tc.tile_pool(name="xr", bufs=1))
        sq_p = ctx.enter_context(tc.tile_pool(name="sq", bufs=2))
        wsq_p = ctx.enter_context(tc.tile_pool(name="wsq", bufs=4))
        w2_p = ctx.enter_context(tc.tile_pool(name="w2", bufs=1))
        x2_p = ctx.enter_context(tc.tile_pool(name="x2", bufs=1))
        o_p = ctx.enter_context(tc.tile_pool(name="o", bufs=4))
        ps_p = ctx.enter_context(tc.tile_pool(name="ps", bufs=6, space="PSUM"))
        psw2_p = ctx.enter_context(tc.tile_pool(name="psw2", bufs=2, space="PSUM"))

        neghalf = const_p.tile([P, P], bf16)
        nc.vector.memset(neghalf[:], -0.5)
        ones8 = const_p.tile([1, 2, P], fp8)    # DoubleRow preload stationary
        nc.vector.memset(ones8[:], 1.0)

        xt_sb = xt_p.tile([P, KT, B], MM_DT)    # x.T resident, matmul stationary
        w_sb = w_p.tile([P, KT, OS], MM_DT)     # w shard resident, matmul moving
        xr_sb = xr_p.tile([P, MT, I], f16)      # x rows (fp16) for x2
        w2pair = w2_p.tile([1, 2, OS], fp8)     # [-w2/2; zeros] preload rhs rows
        x2col = x2_p.tile([P, MT], f32)         # x2 per-partition, one col per m-tile

        xt_src = xt_d.ap().rearrange("(k p) b -> p k b", p=P)    # [128, KT, B]
        w_src = w_d.ap().rearrange("(k p) o -> p k o", p=P)      # [128, KT, OS]
        x_src = x_d.ap().rearrange("(m p) i -> p m i", p=P)      # [128, MT, I]

        def dma_w_chunk(n, split=1):
            ns = slice(n * NB, (n + 1) * NB)
            kstep = KT // split
            for k0 in range(0, KT, kstep):
                nc.sync.dma_start(w_sb[:, k0:k0 + kstep, ns],
                                  w_src[:, k0:k0 + kstep, ns])

        def dma_xt_chunk(c):
            nc.sync.dma_start(xt_sb[:, :, c * NB:(c + 1) * NB],
                              xt_src[:, :, c * NB:(c + 1) * NB])

        def dma_x_rows(m0, m1):
            nc.sync.dma_start(xr_sb[:, m0:m1, :], x_src[:, m0:m1, :])

        # input DMAs, ordered so the PE's earliest dependencies land first:
        # the main loop runs (n-block, m-half) super-blocks, so block 0 only
        # needs w chunk 0 + half of xt + half of x.
        dma_w_chunk(0, split=4)
        dma_xt_chunk(0)
        dma_xt_chunk(1)
        dma_w_chunk(1)
        dma_x_rows(0, 8)
        dma_xt_chunk(2)
        dma_xt_chunk(3)
        dma_w_chunk(2)
        dma_x_rows(8, 16)
        dma_w_chunk(3)

        sq_dt = f32 if MM_DT is f32r else MM_DT
        nc.vector.memset(w2pair[:], 0.0)

        def emit_w2(n):
            # psw2 = -0.5 * colsum(w^2) broadcast across partitions
            ns = slice(n * NB, (n + 1) * NB)
            psw2 = psw2_p.tile([P, NB], f32)
            for k in range(KT):
                wsq = wsq_p.tile([P, NB], bf16)
                nc.vector.tensor_mul(wsq[:], w_sb[:, k, ns].bitcast(sq_dt),
                                     w_sb[:, k, ns].bitcast(sq_dt))
                nc.tensor.matmul(psw2[:], neghalf[:], wsq[:],
                                 start=(k == 0), stop=(k == KT - 1))
            nc.vector.tensor_copy(w2pair[:, 0, ns], psw2[0:1, :])

        blocks = [(n, h) for n in range(NT) for h in range(2)]
        # w2(n) must precede block 2n (first use) but trail its w-chunk DMA:
        w2_at = {0: 0, 1: 1, 3: 2, 5: 3}
        for bi, (n, h) in enumerate(blocks):
            if bi in w2_at:
                emit_w2(w2_at[bi])
            ns = slice(n * NB, (n + 1) * NB)
            osb = None
            for m in range(h * (MT // 2), (h + 1) * (MT // 2)):
                if n == 0:
                    sq = sq_p.tile([P, I], f32)
                    nc.scalar.activation(sq[:], xr_sb[:, m, :], AF.Square,
                                         accum_out=x2col[:, m:m + 1])
                if m % 4 == 0:
                    osb = o_p.tile([P, 4, NB], f32)
                ps = ps_p.tile([P, NB], f32)
                # seed the group with -w2/2 broadcast via a DoubleRow
                # ones-matmul (same perf mode as the data matmuls)
                nc.tensor.matmul(ps[:], ones8[:], w2pair[:, :, ns],
                                 start=True, stop=False, perf_mode=DR,
                                 skip_group_check=True)
                for j in range(KT // 2):
                    nc.tensor.matmul(ps[:],
                                     xt_sb[:, 2 * j:2 * j + 2, m * P:(m + 1) * P],
                                     w_sb[:, 2 * j:2 * j + 2, ns],
                                     start=False, stop=(j == KT // 2 - 1),
                                     perf_mode=DR, skip_group_check=True)
                nc.scalar.activation(osb[:, m % 4, :], ps[:], AF.Sqrt,
                                     bias=x2col[:, m:m + 1], scale=-2.0)
                if m % 4 == 3:
                    g = m // 4
                    dst = out_d.ap()[n, g * 4 * P:(g + 1) * 4 * P, :].rearrange(
                        "(mm p) j -> p mm j", p=P)
                    nc.sync.dma_start(dst, osb[:])


def build(repeats=1):
    nc = bacc.Bacc("TRN2", target_bir_lowering=False, debug=False,
                   num_devices=N_CORES)
    x_d = nc.dram_tensor("x", [B, I], f16, kind="ExternalInput")
    xt_d = nc.dram_tensor("xt", [I, B], MM_DT, kind="ExternalInput")
    w_d = nc.dram_tensor("w", [I, OS], MM_DT, kind="ExternalInput")
    out_d = nc.dram_tensor("out", [NT, B, NB], f32, kind="ExternalOutput")
    with tile.TileContext(nc) as tc:
        for _ in range(repeats):
            _emit_body(nc, tc, x_d, xt_d, w_d, out_d)
    nc.compile()
    return nc


_NC = None


def _mm_np(a):
    """Cast a float32 array to the matmul host dtype."""
    import ml_dtypes
    if MM_DT is f32r:
        return np.ascontiguousarray(a, dtype=np.float32)
    if MM_DT is fp8:
        return np.ascontiguousarray(np.asarray(a).astype(ml_dtypes.float8_e4m3))
    return np.ascontiguousarray(np.asarray(a).astype(ml_dtypes.bfloat16))


def make_in_maps(x, weight):
    x16 = np.ascontiguousarray(x.astype(np.float16))
    xt = _mm_np(x.T)
    return [{"x": x16, "xt": xt,
             "w": _mm_np(weight[:, c * OS:(c + 1) * OS])}
            for c in range(N_CORES)]


def assemble(results):
    cols = []
    for c in range(N_CORES):
        blk = results[c]["out"]          # [NT, B, NB]
        cols.append(blk.transpose(1, 0, 2).reshape(B, OS))
    return np.ascontiguousarray(np.concatenate(cols, axis=1))


def kernel(x, weight):
    global _NC
    x = np.asarray(x, dtype=np.float32)
    weight = np.asarray(weight, dtype=np.float32)
    if _NC is None:
        _NC = build(repeats=1)
    in_maps = make_in_maps(x, weight)
    res = run_bass_kernel_spmd(_NC, in_maps, core_ids=list(range(N_CORES)))
    return assemble(res.results)
